# revision 1
# baseline (speedup 1.0000x reference)
"""KMeansSegmentator kernel for 8 Trainium2 NeuronCores.

Math (per row r = (batch, patch), d=1024, k=64 clusters, 256 pixels/patch):
    scores_j = c2_j - 2 * <feat_r, C_j>          (x2 term dropped: constant in j)
    a        = argmax_j scores_j                 (first occurrence on ties)
    out[r]   = cluster_labels[:, a]              (256 label values)

Device pipeline per core (rows sharded by batch, 16 batches = 3136 rows/core):
    mm1:   scores[64, R] = (-2C)^T_chunks @ featT_chunks   (PE, accumulated over 8 K-chunks)
    +c2:   fused with PSUM->SBUF copy (DVE tensor_scalar add)
    PE-transpose scores -> [rows, 64], exact first-argmax via iota trick (DVE)
    PE-transpose onehot -> [64, rows], mm2: out[rows, 256] = onehot^T @ labelsT
    contiguous DMA out.

Host does the sharding layout (feat transpose per shard) and the final
patch-grid rearrangement; both are part of the unshard/shard contract.
"""

import sys

sys.path.insert(0, "/opt/trn_rl_repo")

import numpy as np

import concourse.bass as bass
import concourse.mybir as mybir
from concourse import tile
from concourse.bass_utils import run_bass_kernel_spmd

N_CORES = 8
BS, NPATCH, D, K = 128, 196, 1024, 64
PIX = 256  # 16*16 pixels per patch
ROWS = (BS // N_CORES) * NPATCH  # 3136 rows per core
GROUP = 512  # rows per matmul group (PSUM bank = 512 fp32)
NCHUNK = D // 128  # 8 contraction chunks

F32 = mybir.dt.float32
# Matmul operand dtypes (flip to float32r for speed once precision verified)
MM1_DT = mybir.dt.float32
MM2_DT = mybir.dt.float32r


def split_waits(nc, cap=1):
    """Walrus in this container rejects >1 sync-wait per instruction; hoist
    excess waits onto same-engine NoOps inserted just before the instruction."""
    n_split = 0
    for bb in nc.main_func.blocks:
        new_insts = []
        for inst in bb.instructions:
            si = inst.sync_info
            if si is not None and si.on_wait and len(si.on_wait) > cap:
                waits = list(si.on_wait)
                chunks = [waits[i : i + cap] for i in range(0, len(waits), cap)]
                for ch in chunks[:-1]:
                    nop = mybir.InstNoOp(
                        name=f"{inst.name}-wsplit{n_split}",
                        engine=inst.engine,
                        ins=[],
                        outs=[],
                        sync_info=mybir.SyncInfo(on_wait=ch, on_update=[]),
                    )
                    n_split += 1
                    new_insts.append(nop)
                si.on_wait = chunks[-1]
            new_insts.append(inst)
        bb.instructions[:] = new_insts
    return nc


def build(rows=ROWS, mm1_dt=MM1_DT, mm2_dt=MM2_DT, repeat=1):
    nc = bass.Bass()
    featT = nc.dram_tensor("featT", [D, rows], mm1_dt, kind="ExternalInput")
    cneg2 = nc.dram_tensor("cneg2", [D, K], mm1_dt, kind="ExternalInput")
    c2 = nc.dram_tensor("c2", [K, 1], F32, kind="ExternalInput")
    labelsT = nc.dram_tensor("labelsT", [K, PIX], mm2_dt, kind="ExternalInput")
    iota = nc.dram_tensor("iota", [128, K], F32, kind="ExternalInput")
    ident = nc.dram_tensor("ident", [128, 128], F32, kind="ExternalInput")
    identm = nc.dram_tensor("identm", [128, 128], mm2_dt, kind="ExternalInput")
    out = nc.dram_tensor("out", [rows, PIX], F32, kind="ExternalOutput")

    groups = []
    r0 = 0
    while r0 < rows:
        groups.append((r0, min(GROUP, rows - r0)))
        r0 += GROUP

    with tile.TileContext(nc) as tc:
        with (
            tc.tile_pool(name="const", bufs=1) as constp,
            tc.tile_pool(name="feat", bufs=3) as featp,
            tc.tile_pool(name="sc", bufs=2) as scp,
            tc.tile_pool(name="small", bufs=3) as smallp,
            tc.tile_pool(name="oh", bufs=3) as ohp,
            tc.tile_pool(name="outsb", bufs=3) as outp,
            tc.tile_pool(name="ps_mm1", bufs=2, space="PSUM") as ps_mm1,
            tc.tile_pool(name="ps_tr", bufs=4, space="PSUM") as ps_tr,
            tc.tile_pool(name="ps_out", bufs=2, space="PSUM") as ps_out,
        ):
            # ---- constants (loaded once) ----
            cneg2_sb = constp.tile([128, NCHUNK, K], mm1_dt)
            nc.sync.dma_start(
                out=cneg2_sb[:], in_=cneg2[:].rearrange("(c p) k -> p c k", p=128)
            )
            c2_sb = constp.tile([K, 1], F32)
            nc.sync.dma_start(out=c2_sb[:], in_=c2[:])
            labelsT_sb = constp.tile([K, PIX], mm2_dt)
            nc.sync.dma_start(out=labelsT_sb[:], in_=labelsT[:])
            iota_sb = constp.tile([128, K], F32)
            nc.sync.dma_start(out=iota_sb[:], in_=iota[:])
            ident_sb = constp.tile([128, 128], F32)
            nc.sync.dma_start(out=ident_sb[:], in_=ident[:])
            identm_sb = constp.tile([128, 128], mm2_dt)
            nc.sync.dma_start(out=identm_sb[:], in_=identm[:])

            for _rep in range(repeat):
                for r0, R in groups:
                    # feat^T tile: [128 dpart, chunk, R rows]
                    ft = featp.tile([128, NCHUNK, R], mm1_dt, tag="ft")
                    nc.sync.dma_start(
                        out=ft[:],
                        in_=featT[:, r0 : r0 + R].rearrange("(c p) r -> p c r", p=128),
                    )
                    # mm1: scores[64, R] accumulated over 8 chunks
                    scores_ps = ps_mm1.tile([K, R], F32, tag="scores_ps")
                    for c in range(NCHUNK):
                        nc.tensor.matmul(
                            scores_ps[:],
                            cneg2_sb[:, c, :],
                            ft[:, c, :],
                            start=(c == 0),
                            stop=(c == NCHUNK - 1),
                        )
                    # +c2 fused with PSUM->SBUF copy
                    scores_sb = scp.tile([K, R], F32, tag="scores_sb")
                    nc.vector.tensor_scalar(
                        scores_sb[:], scores_ps[:], c2_sb[:], None, op0=mybir.AluOpType.add
                    )
                    ntile = (R + 127) // 128
                    for t in range(ntile):
                        T = min(128, R - t * 128)
                        sl = slice(t * 128, t * 128 + T)
                        # scoresT[rows, k]
                        scT_ps = ps_tr.tile([128, K], F32, tag="tr")
                        nc.tensor.transpose(
                            scT_ps[:T, :], scores_sb[:, sl], ident_sb[:K, :K]
                        )
                        # exact first-occurrence argmax -> onehot
                        m_sb = smallp.tile([128, 1], F32, tag="m")
                        nc.vector.reduce_max(
                            out=m_sb[:T, :], in_=scT_ps[:T, :], axis=mybir.AxisListType.X
                        )
                        cand_sb = smallp.tile([128, K], F32, tag="cand")
                        nc.vector.tensor_scalar(
                            cand_sb[:T, :],
                            scT_ps[:T, :],
                            m_sb[:T, :],
                            None,
                            op0=mybir.AluOpType.is_ge,
                        )
                        tv_sb = smallp.tile([128, K], F32, tag="tv")
                        nc.vector.tensor_tensor(
                            out=tv_sb[:T, :],
                            in0=cand_sb[:T, :],
                            in1=iota_sb[:T, :],
                            op=mybir.AluOpType.mult,
                        )
                        tmax_sb = smallp.tile([128, 1], F32, tag="tmax")
                        nc.vector.reduce_max(
                            out=tmax_sb[:T, :], in_=tv_sb[:T, :], axis=mybir.AxisListType.X
                        )
                        onehot_sb = ohp.tile([128, K], mm2_dt, tag="onehot")
                        nc.vector.tensor_scalar(
                            onehot_sb[:T, :],
                            iota_sb[:T, :],
                            tmax_sb[:T, :],
                            None,
                            op0=mybir.AluOpType.is_equal,
                        )
                        # onehot^T[k, rows] for mm2 lhsT
                        ohT_ps = ps_tr.tile([K, 128], mm2_dt, tag="tr")
                        nc.tensor.transpose(
                            ohT_ps[:, :T], onehot_sb[:T, :], identm_sb[:T, :T]
                        )
                        ohT_sb = ohp.tile([K, 128], mm2_dt, tag="ohT_sb")
                        nc.scalar.copy(out=ohT_sb[:, :T], in_=ohT_ps[:, :T])
                        # mm2: out[rows, 256] = onehot^T.T @ labelsT
                        out_ps = ps_out.tile([128, PIX], F32, tag="out_ps")
                        nc.tensor.matmul(
                            out_ps[:T, :],
                            ohT_sb[:, :T],
                            labelsT_sb[:],
                            start=True,
                            stop=True,
                        )
                        out_sb = outp.tile([128, PIX], F32, tag="out_sb")
                        nc.scalar.copy(out=out_sb[:T, :], in_=out_ps[:T, :])
                        nc.sync.dma_start(
                            out=out[r0 + t * 128 : r0 + t * 128 + T, :], in_=out_sb[:T, :]
                        )
    return split_waits(nc)


_NC_CACHE = {}


def _get_nc():
    key = (ROWS, MM1_DT, MM2_DT)
    if key not in _NC_CACHE:
        _NC_CACHE[key] = build()
    return _NC_CACHE[key]


def make_in_maps(feat, centroids, cluster_labels):
    feat = np.ascontiguousarray(np.asarray(feat, np.float32))
    C = np.asarray(centroids, np.float32)
    L = np.asarray(cluster_labels, np.float32)
    consts = {
        "cneg2": np.ascontiguousarray(-2.0 * C),
        "c2": np.ascontiguousarray((C * C).sum(0, dtype=np.float32).reshape(K, 1)),
        "labelsT": np.ascontiguousarray(L.T),
        "iota": np.broadcast_to(
            (K - np.arange(K, dtype=np.float32))[None, :], (128, K)
        ).copy(),
        "ident": np.eye(128, dtype=np.float32),
        "identm": np.eye(128, dtype=np.float32),
    }
    bpc = BS // N_CORES
    in_maps = []
    for core in range(N_CORES):
        shard = feat[core * bpc : (core + 1) * bpc].reshape(bpc * NPATCH, D)
        in_maps.append({"featT": np.ascontiguousarray(shard.T), **consts})
    return in_maps


def assemble(outs):
    pred = np.concatenate(outs, axis=0)  # [25088, 256]
    pred = pred.reshape(BS, 14, 14, 16, 16).transpose(0, 1, 3, 2, 4)
    return np.ascontiguousarray(pred.reshape(BS, 224, 224), dtype=np.float32)


def run(inputs, trace=False, **kw):
    nc = _get_nc()
    in_maps = make_in_maps(
        inputs["feat"], inputs["centroids"], inputs["cluster_labels"]
    )
    res = run_bass_kernel_spmd(nc, in_maps, list(range(N_CORES)), trace=trace, **kw)
    outs = [res.results[c]["out"] for c in range(N_CORES)]
    return assemble(outs), res


def kernel(**inputs):
    out, _ = run(inputs, trace=False)
    return out



# revision 32
# speedup vs baseline: 1.8526x; 1.8526x over previous
"""KMeansSegmentator kernel for 8 Trainium2 NeuronCores.

Math (per row r = (batch, patch), d=1024, k=64 clusters, 256 pixels/patch):
    scores_j = c2_j - 2 * <feat_r, C_j>          (x2 term dropped: constant in j)
    a        = argmax_j scores_j                 (first occurrence on ties)
    out[r]   = cluster_labels[:, a]              (256 label values)

Device pipeline per core (rows sharded by batch, 16 batches = 3136 rows/core,
processed in 25 tiles of 128 rows; tail tile is 64):
    mm1:  scores_ps[T,64] = ones^T@c2row + sum_c ft[:,c,:]^T @ (-2C)[:,c,:]
          Feat tile is the 128-wide stationary operand so the full PE array is
          used and the result lands row-major (no transpose).  fp32 exact; the
          rank-1 init folds the +c2 bias into the PSUM accumulation.
    argmax: DVE sort8 max + max_index (first-occurrence on ties), onehot via
          compare against a broadcast index.
    mm2:  PE-transpose onehot, out[T,256] = onehot^T @ labelsT in fp32r with
          labels pre-scaled by 254; Act copy casts PSUM->uint8 for the output
          DMA (worst-case quantization ~1/254, far inside the 2e-2 gate).
    The PE stream is software-pipelined one tile (mm1 of tile t+1 issues
    before transpose/mm2 of tile t) so the argmax latency doesn't throttle
    the feat DMA, which is the roofline resource.

Host does the sharding layout (feat transpose per shard), un-scales the uint8
output, and does the final patch-grid rearrangement; all part of the
shard/unshard contract.
"""

import sys

sys.path.insert(0, "/opt/trn_rl_repo")

import numpy as np

import concourse.bass as bass
import concourse.mybir as mybir
from concourse import tile
from concourse.bass_utils import run_bass_kernel_spmd

N_CORES = 8
BS, NPATCH, D, K = 128, 196, 1024, 64
PIX = 256  # 16*16 pixels per patch
ROWS = (BS // N_CORES) * NPATCH  # 3136 rows per core
NCHUNK = D // 128  # 8 contraction chunks
TILE = 128
NTILES = (ROWS + TILE - 1) // TILE  # 25 (last tile = 64 rows)
OUTB = 4  # tiles per output DMA batch (steady state)

F32 = mybir.dt.float32
F32R = mybir.dt.float32r
U32 = mybir.dt.uint32
U8 = mybir.dt.uint8
LSCALE = 254.0  # labels pre-scaled by this on host; output uint8, host divides


def split_waits(nc, cap=1):
    """Walrus in this container rejects >1 sync-wait per instruction; hoist
    excess waits onto same-engine NoOps inserted just before the instruction."""
    n_split = 0
    for bb in nc.main_func.blocks:
        new_insts = []
        for inst in bb.instructions:
            si = inst.sync_info
            if si is not None and si.on_wait and len(si.on_wait) > cap:
                waits = list(si.on_wait)
                chunks = [waits[i : i + cap] for i in range(0, len(waits), cap)]
                for ch in chunks[:-1]:
                    nop = mybir.InstNoOp(
                        name=f"{inst.name}-wsplit{n_split}",
                        engine=inst.engine,
                        ins=[],
                        outs=[],
                        sync_info=mybir.SyncInfo(on_wait=ch, on_update=[]),
                    )
                    n_split += 1
                    new_insts.append(nop)
                si.on_wait = chunks[-1]
            new_insts.append(inst)
        bb.instructions[:] = new_insts
    return nc


def _batches(ntiles, pattern=None):
    """Out-batch sizes; one DMA issue chain per batch (fired after the
    batch's last tile copy), so fewer batches = fewer serial issue chains."""
    if pattern is not None:
        assert sum(pattern) == ntiles
        return list(pattern)
    batches = []
    left = ntiles
    while left > 0:
        if left > 6:
            batches.append(OUTB)
            left -= OUTB
        elif left > 3:
            batches.append(2)
            left -= 2
        else:
            batches.append(1)
            left -= 1
    return batches


def build(rows=ROWS, psum_direct=True, tt_oneh=True, pipe=2, taper=True,
          ohT_eng="scalar", outcp_eng="scalar", feat_bufs=6, out_dma="scalar",
          bat_pattern=(7, 7, 7, 4)):
    nc = bass.Bass()
    featT = nc.dram_tensor("featT", [D, rows], F32, kind="ExternalInput")
    cneg2p = nc.dram_tensor("cneg2p", [128, NCHUNK * K], F32, kind="ExternalInput")
    c2row = nc.dram_tensor("c2row", [1, K], F32, kind="ExternalInput")
    ones_col = nc.dram_tensor("ones_col", [1, 128], F32, kind="ExternalInput")
    c2rep = nc.dram_tensor("c2rep", [128, K], F32, kind="ExternalInput")
    iota_u = nc.dram_tensor("iota_u", [128, K], U32, kind="ExternalInput")
    iota_f = nc.dram_tensor("iota_f", [128, K], F32, kind="ExternalInput")
    labelsT = nc.dram_tensor("labelsT", [K, PIX], F32R, kind="ExternalInput")
    identm = nc.dram_tensor("identm", [128, 128], F32R, kind="ExternalInput")
    ntiles = (rows + TILE - 1) // TILE
    # tile-major output layout: out[p, t, x] is row t*128+p. Keeps each
    # DMA descriptor >= 512B (batches of tiles are contiguous per partition).
    out = nc.dram_tensor("out", [TILE, ntiles, PIX], U8, kind="ExternalOutput")

    batches = _batches(ntiles, bat_pattern)
    max_bn = max(batches)
    bat_of_tile, acc = [], 0
    for bi, bn in enumerate(batches):
        for s in range(bn):
            bat_of_tile.append((bi, s, acc))
        acc += bn

    with tile.TileContext(nc) as tc:
        with (
            tc.tile_pool(name="const", bufs=1) as constp,
            tc.tile_pool(name="feat", bufs=feat_bufs) as featp,
            tc.tile_pool(name="sc", bufs=3) as scp,
            tc.tile_pool(name="small", bufs=4) as smallp,
            tc.tile_pool(name="oh", bufs=3) as ohp,
            tc.tile_pool(name="outsb", bufs=2) as outp,
            tc.tile_pool(name="ps_sc", bufs=4, space="PSUM") as ps_sc,
            tc.tile_pool(name="ps_tr", bufs=2, space="PSUM") as ps_tr,
            tc.tile_pool(name="ps_out", bufs=2, space="PSUM") as ps_out,
        ):
            # ---- constants (loaded once, on the Act queue so SP can issue
            # feat-tile DMAs in parallel) ----
            cneg2_sb = constp.tile([128, NCHUNK * K], F32)
            nc.scalar.dma_start(out=cneg2_sb[:], in_=cneg2p[:])
            ones_sb = constp.tile([1, 128], F32)
            nc.scalar.dma_start(out=ones_sb[:], in_=ones_col[:])
            c2row_sb = constp.tile([1, K], F32)
            nc.scalar.dma_start(out=c2row_sb[:], in_=c2row[:])
            if not psum_direct:
                c2rep_sb = constp.tile([128, K], F32)
                nc.scalar.dma_start(out=c2rep_sb[:], in_=c2rep[:])
            if tt_oneh:
                iota_sb = constp.tile([128, K], U32)
                nc.scalar.dma_start(out=iota_sb[:], in_=iota_u[:])
            else:
                iota_sb = constp.tile([128, K], F32)
                nc.scalar.dma_start(out=iota_sb[:], in_=iota_f[:])
            labelsT_sb = constp.tile([K, PIX], F32R)
            nc.scalar.dma_start(out=labelsT_sb[:], in_=labelsT[:])
            identm_sb = constp.tile([128, 128], F32R)
            nc.scalar.dma_start(out=identm_sb[:], in_=identm[:])

            state = {}
            out_dma_eng = getattr(nc, {"gpsimd": "gpsimd", "scalar": "scalar",
                                       "sync": "sync"}[out_dma])

            def front(t):
                r0 = t * TILE
                T = min(TILE, rows - r0)
                ft = featp.tile([128, NCHUNK, TILE], F32, tag="ft")
                nc.sync.dma_start(
                    out=ft[:, :, :T],
                    in_=featT[:, r0 : r0 + T].rearrange("(c p) r -> p c r", p=128),
                )
                ps = ps_sc.tile([TILE, K], F32, tag="ps")
                if psum_direct:
                    # rank-1 c2 bias seeds the accumulation
                    nc.tensor.matmul(
                        ps[:T, :], ones_sb[:, :T], c2row_sb[:], start=True, stop=False
                    )
                for c in range(NCHUNK):
                    nc.tensor.matmul(
                        ps[:T, :],
                        ft[:, c, :T],
                        cneg2_sb[:, c * K : (c + 1) * K],
                        start=(not psum_direct and c == 0),
                        stop=(c == NCHUNK - 1),
                    )
                if psum_direct:
                    sc = ps  # DVE reads scores straight from PSUM
                else:
                    sc = scp.tile([TILE, K], F32, tag="sc")
                    nc.vector.tensor_tensor(
                        out=sc[:T, :], in0=ps[:T, :], in1=c2rep_sb[:T, :],
                        op=mybir.AluOpType.add,
                    )
                m8 = smallp.tile([TILE, 8], F32, tag="m8")
                nc.vector.max(out=m8[:T, :], in_=sc[:T, :])
                ix = smallp.tile([TILE, 8], U32, tag="ix")
                nc.vector.max_index(out=ix[:T, :], in_max=m8[:T, :], in_values=sc[:T, :])
                oh = ohp.tile([TILE, K], F32R, tag="oh")
                if tt_oneh:
                    nc.vector.tensor_tensor(
                        out=oh[:T, :],
                        in0=iota_sb[:T, :],
                        in1=ix[:T, 0:1].broadcast_to([T, K]),
                        op=mybir.AluOpType.is_equal,
                    )
                else:
                    ixf = smallp.tile([TILE, 1], F32, tag="ixf")
                    nc.vector.tensor_copy(out=ixf[:T, :], in_=ix[:T, 0:1])
                    nc.vector.tensor_scalar(
                        out=oh[:T, :],
                        in0=iota_sb[:T, :],
                        scalar1=ixf[:T, :],
                        scalar2=None,
                        op0=mybir.AluOpType.is_equal,
                    )
                state[t] = (oh, T)

            def back(t):
                oh, T = state.pop(t)
                r0 = t * TILE
                ohT_ps = ps_tr.tile([K, TILE], F32R, tag="ohT_ps")
                nc.tensor.transpose(ohT_ps[:, :T], oh[:T, :], identm_sb[:T, :T])
                ohT = ohp.tile([K, TILE], F32R, tag="ohT")
                if ohT_eng == "scalar":
                    nc.scalar.copy(out=ohT[:, :T], in_=ohT_ps[:, :T])
                else:
                    nc.vector.tensor_copy(out=ohT[:, :T], in_=ohT_ps[:, :T])
                op_ = ps_out.tile([TILE, PIX], F32, tag="op")
                nc.tensor.matmul(
                    op_[:T, :], ohT[:, :T], labelsT_sb[:], start=True, stop=True
                )
                b, s, b0 = bat_of_tile[t]
                bn = batches[b]
                if s == 0:
                    state[("ob", b)] = outp.tile(
                        [TILE, max_bn, PIX], U8, tag="ob", name=f"ob{b}"
                    )
                ob = state[("ob", b)]
                if outcp_eng == "scalar":
                    nc.scalar.copy(out=ob[:T, s, :], in_=op_[:T, :])
                elif outcp_eng == "vector":
                    nc.vector.tensor_copy(out=ob[:T, s, :], in_=op_[:T, :])
                else:
                    nc.gpsimd.tensor_copy(out=ob[:T, s, :], in_=op_[:T, :])
                if s == bn - 1:
                    # full 128 partitions even when the batch's last tile is
                    # short: other tiles in the batch need rows T..127, and the
                    # short tile's extra rows land past ROWS (host drops them)
                    out_dma_eng.dma_start(
                        out=out[:, b0 : b0 + bn, :], in_=ob[:, :bn, :]
                    )
                    del state[("ob", b)]

            # interleave fronts and backs with `pipe` tiles of skew; taper the
            # skew near the end so fewer back stages drain after the last front
            sched = []
            emitted = 0
            for t in range(ntiles):
                sched.append(("f", t))
                lag = pipe if (not taper or t < ntiles - 1) else 1
                while emitted <= t - lag:
                    sched.append(("b", emitted))
                    emitted += 1
            while emitted < ntiles:
                sched.append(("b", emitted))
                emitted += 1
            for kind, t in sched:
                (front if kind == "f" else back)(t)
    return split_waits(nc)


_NC_CACHE = {}


def _get_nc():
    if "nc" not in _NC_CACHE:
        _NC_CACHE["nc"] = build()
    return _NC_CACHE["nc"]


def make_in_maps(feat, centroids, cluster_labels):
    feat = np.ascontiguousarray(np.asarray(feat, np.float32))
    C = np.asarray(centroids, np.float32)
    L = np.asarray(cluster_labels, np.float32)
    c2 = (C * C).sum(0, dtype=np.float32)
    consts = {
        "cneg2p": np.ascontiguousarray(
            (-2.0 * C).reshape(NCHUNK, 128, K).transpose(1, 0, 2).reshape(128, NCHUNK * K)
        ),
        "c2row": np.ascontiguousarray(c2[None, :]),
        "ones_col": np.ones((1, 128), dtype=np.float32),
        "c2rep": np.ascontiguousarray(np.broadcast_to(c2[None, :], (128, K))),
        "iota_u": np.ascontiguousarray(
            np.broadcast_to(np.arange(K, dtype=np.uint32)[None, :], (128, K))
        ),
        "iota_f": np.ascontiguousarray(
            np.broadcast_to(np.arange(K, dtype=np.float32)[None, :], (128, K))
        ),
        "labelsT": np.ascontiguousarray(L.T * LSCALE),
        "identm": np.eye(128, dtype=np.float32),
    }
    bpc = BS // N_CORES
    in_maps = []
    for core in range(N_CORES):
        shard = feat[core * bpc : (core + 1) * bpc].reshape(bpc * NPATCH, D)
        in_maps.append({"featT": np.ascontiguousarray(shard.T), **consts})
    return in_maps


def assemble(outs):
    # outs are tile-major [128, ntiles, 256]; row t*128+p -> [rows, 256]
    rows = []
    for o in outs:
        r = np.asarray(o, np.float32).transpose(1, 0, 2).reshape(-1, PIX)
        rows.append(r[:ROWS] / LSCALE)
    pred = np.concatenate(rows, axis=0)
    pred = pred.reshape(BS, 14, 14, 16, 16).transpose(0, 1, 3, 2, 4)
    return np.ascontiguousarray(pred.reshape(BS, 224, 224), dtype=np.float32)


def run(inputs, trace=False, **kw):
    nc = _get_nc()
    in_maps = make_in_maps(
        inputs["feat"], inputs["centroids"], inputs["cluster_labels"]
    )
    res = run_bass_kernel_spmd(nc, in_maps, list(range(N_CORES)), trace=trace, **kw)
    outs = [res.results[c]["out"] for c in range(N_CORES)]
    return assemble(outs), res


def kernel(**inputs):
    out, _ = run(inputs, trace=False)
    return out


# revision 55
# speedup vs baseline: 1.8756x; 1.0124x over previous
"""KMeansSegmentator kernel for 8 Trainium2 NeuronCores.

Math (per row r = (batch, patch), d=1024, k=64 clusters, 256 pixels/patch):
    scores_j = c2_j - 2 * <feat_r, C_j>          (x2 term dropped: constant in j)
    a        = argmax_j scores_j                 (first occurrence on ties)
    out[r]   = cluster_labels[:, a]              (256 label values)

Device pipeline per core (rows sharded by batch, 16 batches = 3136 rows/core,
processed in 25 tiles of 128 rows; tail tile is 64):
    mm1:  scores_ps[T,64] = ones^T@c2row + sum_c ft[:,c,:]^T @ (-2C)[:,c,:]
          Feat tile is the 128-wide stationary operand so the full PE array is
          used and the result lands row-major (no transpose).  fp32 exact; the
          rank-1 init folds the +c2 bias into the PSUM accumulation.
    argmax: DVE sort8 max + max_index straight from PSUM (first-occurrence on
          ties), onehot via u32 compare against the broadcast top-1 index.
    mm2:  PE-transpose onehot, out[T,256] = onehot^T @ labelsT in fp32r with
          labels pre-scaled by 254; Act copy casts PSUM->uint8 for the output
          DMA (worst-case quantization ~1/254, far inside the 2e-2 gate).
    The PE stream is software-pipelined two tiles (mm1 of tile t+2 issues
    before transpose/mm2 of tile t) so the argmax latency doesn't throttle
    the feat DMA, which is the roofline resource; the skew tapers to one at
    the tail to shorten the drain.  All constants arrive in one packed DMA.

Host does the sharding layout (feat transpose per shard), un-scales the uint8
output, and does the final patch-grid rearrangement; all part of the
shard/unshard contract.
"""

import sys

sys.path.insert(0, "/opt/trn_rl_repo")

import numpy as np

import concourse.bass as bass
import concourse.mybir as mybir
from concourse import tile
from concourse.bass_utils import run_bass_kernel_spmd

N_CORES = 8
BS, NPATCH, D, K = 128, 196, 1024, 64
PIX = 256  # 16*16 pixels per patch
ROWS = (BS // N_CORES) * NPATCH  # 3136 rows per core
NCHUNK = D // 128  # 8 contraction chunks
TILE = 128
NTILES = (ROWS + TILE - 1) // TILE  # 25 (last tile = 64 rows)

F32 = mybir.dt.float32
F32R = mybir.dt.float32r
U32 = mybir.dt.uint32
U8 = mybir.dt.uint8
LSCALE = 254.0  # labels pre-scaled by this on host; output uint8, host divides

# packed constant layouts (words per partition). Two packs because the BIR
# verifier requires fp32r matmul operands to be produced as fp32r — so the
# fp32r-consumed constants arrive via their own fp32r-typed DMA.
_CN0, _CN1 = 0, 512          # cneg2p [128, 8*64]           (f32 pack)
_IO0, _IO1 = 512, 576        # iota u32 [128, 64]
_C20, _C21 = 576, 640        # c2 row [1, 64]
_ON0, _ON1 = 640, 768        # ones row [1, 128]
_CR0, _CR1 = 768, 832        # c2 replicated [128, 64]
CPACK = _CR1
_LB0, _LB1 = 0, 256          # labelsT [64, 256]            (f32r pack)
_ID0, _ID1 = 256, 384        # identity [128, 128]
CPACKR = _ID1


def split_waits(nc, cap=1):
    """Walrus in this container rejects >1 sync-wait per instruction; hoist
    excess waits onto same-engine NoOps inserted just before the instruction."""
    n_split = 0
    for bb in nc.main_func.blocks:
        new_insts = []
        for inst in bb.instructions:
            si = inst.sync_info
            if si is not None and si.on_wait and len(si.on_wait) > cap:
                waits = list(si.on_wait)
                chunks = [waits[i : i + cap] for i in range(0, len(waits), cap)]
                for ch in chunks[:-1]:
                    nop = mybir.InstNoOp(
                        name=f"{inst.name}-wsplit{n_split}",
                        engine=inst.engine,
                        ins=[],
                        outs=[],
                        sync_info=mybir.SyncInfo(on_wait=ch, on_update=[]),
                    )
                    n_split += 1
                    new_insts.append(nop)
                si.on_wait = chunks[-1]
            new_insts.append(inst)
        bb.instructions[:] = new_insts
    return nc


def build(rows=ROWS, pipe=2, taper=0, feat_bufs=5, bat_pattern=(6, 6, 6, 6, 1),
          tail_dma_sync=True, tail_cp_vec=0, c2_dve=False, sc_bufs=4,
          oh_bufs=4, small_bufs=4, cpack_eng="scalar"):
    nc = bass.Bass()
    featT = nc.dram_tensor("featT", [D, rows], F32, kind="ExternalInput")
    cpack = nc.dram_tensor("cpack", [128, CPACK], F32, kind="ExternalInput")
    cpackr = nc.dram_tensor("cpackr", [128, CPACKR], F32R, kind="ExternalInput")
    ntiles = (rows + TILE - 1) // TILE
    # tile-major output layout: out[p, t, x] is row t*128+p. Keeps each
    # DMA descriptor >= 512B (batches of tiles are contiguous per partition).
    out = nc.dram_tensor("out", [TILE, ntiles, PIX], U8, kind="ExternalOutput")

    batches = list(bat_pattern) if sum(bat_pattern) == ntiles else None
    if batches is None:
        batches, left = [], ntiles
        while left > 0:
            take = min(6, left) if left > 1 else 1
            if left - take == 0 and take > 1:
                take -= 1
            batches.append(take)
            left -= take
    max_bn = max(batches)
    bat_of_tile, acc = [], 0
    for bi, bn in enumerate(batches):
        for s in range(bn):
            bat_of_tile.append((bi, s, acc))
        acc += bn

    with tile.TileContext(nc) as tc:
        with (
            tc.tile_pool(name="const", bufs=1) as constp,
            tc.tile_pool(name="feat", bufs=feat_bufs) as featp,
            tc.tile_pool(name="small", bufs=small_bufs) as smallp,
            tc.tile_pool(name="oh", bufs=oh_bufs) as ohp,
            tc.tile_pool(name="outsb", bufs=2) as outp,
            tc.tile_pool(name="ps_sc", bufs=sc_bufs, space="PSUM") as ps_sc,
            tc.tile_pool(name="ps_tr", bufs=2, space="PSUM") as ps_tr,
            tc.tile_pool(name="ps_out", bufs=2, space="PSUM") as ps_out,
        ):
            # ---- all constants in one packed DMA (Act queue; SP issues feat
            # tiles in parallel) ----
            cpk = constp.tile([128, CPACK], F32)
            getattr(nc, cpack_eng).dma_start(out=cpk[:], in_=cpack[:])
            cpkr = constp.tile([128, CPACKR], F32R)
            getattr(nc, cpack_eng).dma_start(out=cpkr[:], in_=cpackr[:])
            cneg2_sb = cpk[:, _CN0:_CN1]
            iota_sb = cpk[:, _IO0:_IO1].bitcast(U32)
            labelsT_sb = cpkr[:K, _LB0:_LB1]
            identm_sb = cpkr[:, _ID0:_ID1]
            c2row_sb = cpk[0:1, _C20:_C21]
            ones_sb = cpk[0:1, _ON0:_ON1]
            c2rep_sb = cpk[:, _CR0:_CR1]

            state = {}

            def front(t):
                r0 = t * TILE
                T = min(TILE, rows - r0)
                ft = featp.tile([128, NCHUNK, TILE], F32, tag="ft")
                nc.sync.dma_start(
                    out=ft[:, :, :T],
                    in_=featT[:, r0 : r0 + T].rearrange("(c p) r -> p c r", p=128),
                )
                ps = ps_sc.tile([TILE, K], F32, tag="ps")
                if not c2_dve:
                    # rank-1 c2 bias seeds the accumulation
                    nc.tensor.matmul(
                        ps[:T, :], ones_sb[:, :T], c2row_sb[:], start=True, stop=False
                    )
                for c in range(NCHUNK):
                    nc.tensor.matmul(
                        ps[:T, :],
                        ft[:, c, :T],
                        cneg2_sb[:, c * K : (c + 1) * K],
                        start=(c2_dve and c == 0),
                        stop=(c == NCHUNK - 1),
                    )
                if c2_dve:
                    sc = ohp.tile([TILE, K], F32, tag="sc")
                    nc.vector.tensor_tensor(
                        out=sc[:T, :], in0=ps[:T, :], in1=c2rep_sb[:T, :],
                        op=mybir.AluOpType.add,
                    )
                else:
                    sc = ps
                m8 = smallp.tile([TILE, 8], F32, tag="m8")
                nc.vector.max(out=m8[:T, :], in_=sc[:T, :])
                ix = smallp.tile([TILE, 8], U32, tag="ix")
                nc.vector.max_index(out=ix[:T, :], in_max=m8[:T, :], in_values=sc[:T, :])
                oh = ohp.tile([TILE, K], F32R, tag="oh")
                nc.vector.tensor_tensor(
                    out=oh[:T, :],
                    in0=iota_sb[:T, :],
                    in1=ix[:T, 0:1].broadcast_to([T, K]),
                    op=mybir.AluOpType.is_equal,
                )
                state[t] = (oh, T)

            def back(t):
                oh, T = state.pop(t)
                ohT_ps = ps_tr.tile([K, TILE], F32R, tag="ohT_ps")
                nc.tensor.transpose(ohT_ps[:, :T], oh[:T, :], identm_sb[:T, :T])
                ohT = ohp.tile([K, TILE], F32R, tag="ohT")
                nc.scalar.copy(out=ohT[:, :T], in_=ohT_ps[:, :T])
                op_ = ps_out.tile([TILE, PIX], F32, tag="op")
                nc.tensor.matmul(
                    op_[:T, :], ohT[:, :T], labelsT_sb[:], start=True, stop=True
                )
                b, s, b0 = bat_of_tile[t]
                bn = batches[b]
                if s == 0:
                    state[("ob", b)] = outp.tile(
                        [TILE, max_bn, PIX], U8, tag="ob", name=f"ob{b}"
                    )
                ob = state[("ob", b)]
                if tail_cp_vec and t >= ntiles - tail_cp_vec:
                    nc.vector.tensor_copy(out=ob[:T, s, :], in_=op_[:T, :])
                else:
                    nc.scalar.copy(out=ob[:T, s, :], in_=op_[:T, :])
                if s == bn - 1:
                    # full 128 partitions even when the batch's last tile is
                    # short: other tiles in the batch need rows T..127, and the
                    # short tile's extra rows land past ROWS (host drops them)
                    eng = (
                        nc.sync
                        if (tail_dma_sync and b >= len(batches) - 2)
                        else nc.scalar
                    )
                    eng.dma_start(out=out[:, b0 : b0 + bn, :], in_=ob[:, :bn, :])
                    del state[("ob", b)]

            # interleave fronts and backs with `pipe` tiles of skew, tapering
            # to a skew of 1 for the last `taper` tiles to shorten the drain
            emitted = 0
            for t in range(ntiles):
                front(t)
                lag = pipe if t < ntiles - taper else 1
                while emitted <= t - lag:
                    back(emitted)
                    emitted += 1
            while emitted < ntiles:
                back(emitted)
                emitted += 1
    return split_waits(nc)


_NC_CACHE = {}


def _get_nc():
    if "nc" not in _NC_CACHE:
        _NC_CACHE["nc"] = build()
    return _NC_CACHE["nc"]


def make_cpack(C, L):
    c2 = (C * C).sum(0, dtype=np.float32)
    pk = np.zeros((128, CPACK), dtype=np.float32)
    pk[:, _CN0:_CN1] = (
        (-2.0 * C).reshape(NCHUNK, 128, K).transpose(1, 0, 2).reshape(128, NCHUNK * K)
    )
    pk[:, _IO0:_IO1] = np.broadcast_to(
        np.arange(K, dtype=np.uint32)[None, :], (128, K)
    ).view(np.float32)
    pk[0, _C20:_C21] = c2
    pk[0, _ON0:_ON1] = 1.0
    pk[:, _CR0:_CR1] = c2[None, :]
    pkr = np.zeros((128, CPACKR), dtype=np.float32)
    pkr[:K, _LB0:_LB1] = L.T * LSCALE
    pkr[:, _ID0:_ID1] = np.eye(128, dtype=np.float32)
    return np.ascontiguousarray(pk), np.ascontiguousarray(pkr)


def make_in_maps(feat, centroids, cluster_labels):
    feat = np.ascontiguousarray(np.asarray(feat, np.float32))
    C = np.asarray(centroids, np.float32)
    L = np.asarray(cluster_labels, np.float32)
    cpk, cpkr = make_cpack(C, L)
    bpc = BS // N_CORES
    in_maps = []
    for core in range(N_CORES):
        shard = feat[core * bpc : (core + 1) * bpc].reshape(bpc * NPATCH, D)
        in_maps.append(
            {"featT": np.ascontiguousarray(shard.T), "cpack": cpk, "cpackr": cpkr}
        )
    return in_maps


def assemble(outs):
    # outs are tile-major [128, ntiles, 256]; row t*128+p -> [rows, 256]
    rows = []
    for o in outs:
        r = np.asarray(o, np.float32).transpose(1, 0, 2).reshape(-1, PIX)
        rows.append(r[:ROWS] / LSCALE)
    pred = np.concatenate(rows, axis=0)
    pred = pred.reshape(BS, 14, 14, 16, 16).transpose(0, 1, 3, 2, 4)
    return np.ascontiguousarray(pred.reshape(BS, 224, 224), dtype=np.float32)


def run(inputs, trace=False, **kw):
    nc = _get_nc()
    in_maps = make_in_maps(
        inputs["feat"], inputs["centroids"], inputs["cluster_labels"]
    )
    res = run_bass_kernel_spmd(nc, in_maps, list(range(N_CORES)), trace=trace, **kw)
    outs = [res.results[c]["out"] for c in range(N_CORES)]
    return assemble(outs), res


def kernel(**inputs):
    out, _ = run(inputs, trace=False)
    return out
